# revision 11
# baseline (speedup 1.0000x reference)
"""Trainium2 Bass kernel for a quantized KAN layer (B-spline MLP).

  out[b,o] = x @ base_weight.T + einsum('bic,oic->bo', bspline_basis(x), round(32*w)/32)

Strategy (8 NeuronCores, contraction/i-sharded):
  - Each core owns a 256-wide slice of the 2048 input features. It computes the
    cubic B-spline basis for its slice on DVE/ACT (closed form:
    basis_c(x) = relu((2-|t|)*s2)^3 - relu((1-|t|)*s1)^3, t=(x-center_c)/h),
    quantizes its weight slice on-device (fp32 magic-number round, bit-exact
    round-half-even), folds base_weight in as a 9th channel, and runs the
    K=2304 x M=4096 x N=2048 matmul in bf16 on the tensor engine.
  - Host sums the 8 partial [4096, 2048] outputs (contraction reduce).
"""

import numpy as np

B, IN, OUT = 4096, 2048, 2048
NCORES = 8
ISH = IN // NCORES          # 256 input features per core
P = 128
NT = ISH // P               # 2 i-tiles per core
NCH = 8                     # spline channels
KT = NT * NCH + NT          # 18 k-tiles (16 spline + 2 base)
NB = 256                    # batch chunk
NCHUNK = B // NB            # 16
NOC = 4                     # output chunks per matmul sweep
OCW = OUT // NOC            # 512 (one PSUM bank per matmul)
MAGIC = 12582912.0          # 1.5 * 2**23, fp32 round-to-int magic
S2 = float((1.0 / 6.0) ** (1.0 / 3.0))
S1 = float((4.0 / 6.0) ** (1.0 / 3.0))

_BUILT = {}


def _build(h, repeat=1):
    from concourse import bacc, bass, mybir, tile

    f32 = mybir.dt.float32
    bf16 = mybir.dt.bfloat16
    AF = mybir.ActivationFunctionType

    nc = bacc.Bacc("TRN2", target_bir_lowering=False, debug=False)

    xt = nc.dram_tensor("xt", [ISH, B], f32, kind="ExternalInput")
    w9 = nc.dram_tensor("w9", [KT * P, OUT], f32, kind="ExternalInput")
    gt = nc.dram_tensor("gt", [P, NCH, NB], f32, kind="ExternalInput")
    outp = nc.dram_tensor("outp", [B, OUT], f32, kind="ExternalOutput")

    with tile.TileContext(nc) as tc:
        with (
            tc.tile_pool(name="const", bufs=1) as cpool,
            tc.tile_pool(name="wres", bufs=1) as wpool,
            tc.tile_pool(name="wstream", bufs=3) as spool,
            tc.tile_pool(name="xin", bufs=3) as xpool,
            tc.tile_pool(name="tmp", bufs=2) as tpool,
            tc.tile_pool(name="bas", bufs=3) as bpool,
            tc.tile_pool(name="outsb", bufs=2) as opool,
            tc.tile_pool(name="psum", bufs=2, space=bass.MemorySpace.PSUM) as ppool,
        ):
            gtile = cpool.tile([P, NCH, NB], f32)
            nc.sync.dma_start(gtile[:], gt[:])

            def bias_const(val):
                t = cpool.tile([P, 1], f32, tag=f"bc{val}")
                nc.vector.memset(t[:], float(val))
                return t

            b_magic = bias_const(MAGIC)
            b_unmag = bias_const(-MAGIC / 32.0)
            b_2s2 = bias_const(2.0 * S2)
            b_s1 = bias_const(S1)

            # Resident quantized weights: [128, KT, OUT] bf16 (72KB/partition).
            qw = wpool.tile([P, KT, OUT], bf16)
            for k in range(KT):
                wraw = spool.tile([P, OUT], f32, tag="wraw")
                nc.sync.dma_start(wraw[:], w9[k * P:(k + 1) * P, :])
                if k < NT * NCH:
                    # round(32w)/32 exactly: fp32 RNE via magic constant.
                    # Alternate engines so the prologue runs on ACT+DVE in
                    # parallel (PE is idle until all of qw is resident).
                    if k % 2 == 0:
                        nc.scalar.activation(wraw[:], wraw[:], AF.Identity,
                                             bias=b_magic[:], scale=32.0)
                        nc.scalar.activation(qw[:, k, :], wraw[:], AF.Identity,
                                             bias=b_unmag[:], scale=1.0 / 32.0)
                    else:
                        nc.vector.tensor_scalar(wraw[:], wraw[:], 32.0, MAGIC,
                                                mybir.AluOpType.mult,
                                                mybir.AluOpType.add)
                        nc.vector.tensor_scalar(qw[:, k, :], wraw[:],
                                                1.0 / 32.0, -MAGIC / 32.0,
                                                mybir.AluOpType.mult,
                                                mybir.AluOpType.add)
                else:
                    nc.scalar.copy(qw[:, k, :], wraw[:])

            for ch in [c for _ in range(repeat) for c in range(NCHUNK)]:
                basis = []
                xcast = []
                for t in range(NT):
                    xc = xpool.tile([P, NB], f32, tag="xc")
                    nc.sync.dma_start(
                        xc[:], xt[t * P:(t + 1) * P, ch * NB:(ch + 1) * NB])
                    xcb = xpool.tile([P, NB], bf16, tag="xcb")
                    nc.vector.tensor_copy(xcb[:], xc[:])
                    xcast.append(xcb)

                    # stacked [128 i, 8 c, 256 b] elementwise chain
                    a = tpool.tile([P, NCH, NB], f32, tag="ta")
                    x8 = xc[:].unsqueeze(1).broadcast_to([P, NCH, NB])
                    nc.vector.tensor_sub(a[:], x8, gtile[:])
                    nc.scalar.activation(a[:], a[:], AF.Abs)
                    r2 = tpool.tile([P, NCH, NB], f32, tag="tr2")
                    nc.scalar.activation(r2[:], a[:], AF.Relu,
                                         bias=b_2s2[:], scale=-S2 / h)
                    r1 = tpool.tile([P, NCH, NB], f32, tag="tr1")
                    nc.scalar.activation(r1[:], a[:], AF.Relu,
                                         bias=b_s1[:], scale=-S1 / h)
                    q = tpool.tile([P, NCH, NB], f32, tag="tq")
                    nc.scalar.activation(q[:], r2[:], AF.Square)
                    nc.vector.tensor_mul(r2[:], q[:], r2[:])
                    nc.scalar.activation(q[:], r1[:], AF.Square)
                    nc.vector.tensor_mul(r1[:], q[:], r1[:])
                    bt_ = bpool.tile([P, NCH, NB], bf16, tag="bas")
                    nc.vector.tensor_sub(bt_[:], r2[:], r1[:])
                    basis.append(bt_)

                for bt in range(NB // P):
                    ps = ppool.tile([P, OUT], f32, tag="ps")
                    for k in range(KT):
                        if k < NT * NCH:
                            t, c = divmod(k, NCH)
                            lhsT = basis[t][:, c, bt * P:(bt + 1) * P]
                        else:
                            lhsT = xcast[k - NT * NCH][:, bt * P:(bt + 1) * P]
                        for oc in range(NOC):
                            nc.tensor.matmul(
                                ps[:, oc * OCW:(oc + 1) * OCW],
                                lhsT,
                                qw[:, k, oc * OCW:(oc + 1) * OCW],
                                start=(k == 0),
                                stop=(k == KT - 1),
                            )
                    osb = opool.tile([P, OUT], f32, tag="osb")
                    nc.vector.tensor_copy(osb[:], ps[:])
                    nc.sync.dma_start(
                        outp[ch * NB + bt * P: ch * NB + (bt + 1) * P, :], osb[:])

    nc.compile()
    return nc


def _stage(x, base_weight, spline_weight, grid):
    """Per-core input staging (shard + layout only; all math is on-device)."""
    centers = grid[0, :NCH] + 2.0 * (grid[0, 1] - grid[0, 0])
    gfull = np.ascontiguousarray(
        np.broadcast_to(centers.astype(np.float32)[None, :, None], (P, NCH, NB)))
    in_maps = []
    for j in range(NCORES):
        sh = slice(j * ISH, (j + 1) * ISH)
        xt = np.ascontiguousarray(x[:, sh].T)
        sw = spline_weight[:, sh, :]                       # [2048, 256, 8]
        sw_r = np.ascontiguousarray(
            sw.reshape(OUT, NT, P, NCH).transpose(1, 3, 2, 0).reshape(NT * NCH * P, OUT))
        base_r = np.ascontiguousarray(base_weight[:, sh].T)  # [256, 2048]
        w9 = np.concatenate([sw_r, base_r], axis=0)
        in_maps.append({"xt": xt, "w9": w9, "gt": gfull})
    return in_maps


def kernel(x, base_weight, spline_weight, grid, _profile=None):
    from concourse import bass_utils

    x = np.asarray(x, dtype=np.float32)
    base_weight = np.asarray(base_weight, dtype=np.float32)
    spline_weight = np.asarray(spline_weight, dtype=np.float32)
    grid = np.asarray(grid, dtype=np.float32)

    h = float(grid[0, 1] - grid[0, 0])
    key = round(h, 9)
    if key not in _BUILT:
        _BUILT[key] = _build(h)
    nc = _BUILT[key]

    in_maps = _stage(x, base_weight, spline_weight, grid)
    kw = {}
    if _profile is not None:
        kw = _profile
    res = bass_utils.run_bass_kernel_spmd(
        nc, in_maps, core_ids=list(range(NCORES)), **kw)

    out = np.zeros((B, OUT), dtype=np.float32)
    for om in res.results:
        out += np.asarray(om["outp"], dtype=np.float32)
    if _profile is not None:
        kernel._last_result = res
    return out
